# revision 19
# baseline (speedup 1.0000x reference)
"""Farthest-point-sampling (npoint=2) Bass kernel for Trainium2 — v2.

Problem: xyz [1, 64, 3, 262144] fp32 -> indices [64, 2] (int64 on host).
Per batch b:
  idx0 = argmax_n y[n]            (y = coord plane 1)
  c    = (x,y,z)[idx0]
  idx1 = argmax_n ((x-cx)^2 + (y-cy)^2 + (z-cz)^2)
argmax = first occurrence on ties (jnp.argmax semantics).

Sharding: data-parallel over batch; 8 NeuronCores x 8 batches each.

v2 design (DMA-paced, ~2x over max8/find_index8 baseline):
- Interleaved per-batch DMA (y plane then x+z planes) on the sync queue.
- y argmax: one segmented VectorE reduce [128,2048]->[128,128segs of 16],
  then an exact two-level locate: per-partition first-max-segment code
  (GpSimd eq/mult vs per-partition rowmax, weights 2048-16s), PE transpose
  of (rowmax, basecode) pair, cross-partition first-occurrence base code,
  then a 16-element indirect-DMA gather of the winning run from HBM and a
  single-partition first-occurrence scan. All compares are exact fp32.
- distance: ScalarE Square(x + (-c)) (bit-exact vs IEEE), PE identity-
  matmul accumulation of the three squares into PSUM (bit-exact fp32
  adds), segmented VectorE reduce from PSUM, same locate/rescue (the
  rescue recomputes the 16 candidate distances from HBM with the same
  ops, bit-identical to the pipeline and to the fp32 reference).
- GpSimd: only standard-library ops (iota, tensor_tensor) + indirect DMA.
"""

import numpy as np

import concourse.bacc as bacc
import concourse.bass as bass
import concourse.mybir as mybir
from concourse.masks import make_identity
from concourse.tile import TileContext

B = 64  # full batch
N_CORES = 8
BPC = B // N_CORES  # batches per core
N = 262144
P = 128
COLS = N // P       # 2048
SEG = 16            # points per segment
NSEG = COLS // SEG  # 128 segments per partition
QCOLS = 512         # matmul moving max (fp32) / PSUM bank
NQ = COLS // QCOLS  # 4 quarters

F32 = mybir.dt.float32
U32 = mybir.dt.uint32
I32 = mybir.dt.int32
AX = mybir.AxisListType.X
OP = mybir.AluOpType
SQUARE = mybir.ActivationFunctionType.Square
IDENT = mybir.ActivationFunctionType.Identity


def build_nc():
    nc = bacc.Bacc()
    xin = nc.dram_tensor("xyz", [BPC, 3, N], F32, kind="ExternalInput")
    out = nc.dram_tensor("idx", [1, 2 * BPC], I32, kind="ExternalOutput")
    xflat = xin.rearrange("b c n -> (b c n)")[:, None]

    with TileContext(nc) as tc:
        with (
            tc.tile_pool(name="consts", bufs=1) as consts,
            tc.tile_pool(name="acc", bufs=1) as acc,
            tc.tile_pool(name="ypool", bufs=BPC) as ypool,
            tc.tile_pool(name="xzpool", bufs=3) as xzpool,
            tc.tile_pool(name="sqpool", bufs=2) as sqpool,
            tc.tile_pool(name="segpool", bufs=2) as segpool,
            tc.tile_pool(name="finpool", bufs=3) as finpool,
            tc.tile_pool(name="psd", bufs=1, space="PSUM") as psd_pool,
            tc.tile_pool(name="pssm", bufs=2, space="PSUM") as pssm,
        ):
            # ---------------- constants ----------------
            ident = consts.tile([P, P], F32)
            make_identity(nc, ident)

            def iota_f32(shape, tag, pattern, base, cm):
                ti = consts.tile(shape, I32, tag=tag + "_i")
                nc.gpsimd.iota(ti, pattern=pattern, base=base, channel_multiplier=cm)
                tf = consts.tile(shape, F32, tag=tag)
                nc.vector.tensor_copy(tf, ti)
                return tf

            # ws[p, s] = 2048 - 16*s  (same every partition)
            ws = iota_f32([P, NSEG], "ws", [[-SEG, NSEG]], COLS, 0)
            # wpb[p] = N - 2048 - 2048*p
            wpb = iota_f32([P, 1], "wpb", [[0, 1]], N - COLS, -COLS)
            # wj16[0, j] = 16 - j
            wj16 = iota_f32([1, SEG], "wj16", [[-1, SEG]], SEG, 0)
            # j16col[j] = j
            j16col = iota_f32([SEG, 1], "j16col", [[0, 1]], 0, 1)
            # cn3val[c] = c*N
            cn3val = iota_f32([3, 1], "cn3val", [[0, 1]], 0, N)
            # c3idx[c] = c
            c3idx = iota_f32([3, 1], "c3idx", [[0, 1]], 0, 1)
            # p48[p] = p over 48 partitions (unused directly, kept for clarity)

            ones16 = consts.tile([1, SEG], F32)
            nc.vector.memset(ones16, 1.0)
            ones48 = consts.tile([1, 48], F32)
            nc.vector.memset(ones48, 1.0)
            ones3 = consts.tile([1, 3], F32)
            nc.vector.memset(ones3, 1.0)
            ones128 = consts.tile([1, P], F32)
            nc.vector.memset(ones128, 1.0)
            cN_1 = consts.tile([1, 1], F32)
            nc.vector.memset(cN_1, float(N))
            c16_1 = consts.tile([1, 1], F32)
            nc.vector.memset(c16_1, float(SEG))
            zero3 = consts.tile([3, 1], F32)
            nc.vector.memset(zero3, 0.0)

            # --- selector matrices for const48 (partition p=3j+c mapping) ---
            # fm3[p, f] = f % 3 on [3,48];  fdiv3[p, f] = f // 3 on [16,48]
            fm3 = iota_f32([3, 16, 3], "fm3", [[0, 16], [1, 3]], 0, 0)
            fdiv3 = iota_f32([16, 16, 3], "fdiv3", [[1, 16], [0, 3]], 0, 0)
            pidx3 = iota_f32([3, 1], "pidx3", [[0, 1]], 0, 1)
            pidx16 = iota_f32([16, 1], "pidx16", [[0, 1]], 0, 1)
            wsel = consts.tile([3, 48], F32)
            nc.vector.tensor_tensor(
                out=wsel, in0=fm3.rearrange("p a b -> p (a b)"),
                in1=pidx3.to_broadcast([3, 48]), op=OP.is_equal,
            )
            wjsel = consts.tile([16, 48], F32)
            nc.vector.tensor_tensor(
                out=wjsel, in0=fdiv3.rearrange("p a b -> p (a b)"),
                in1=pidx16.to_broadcast([16, 48]), op=OP.is_equal,
            )
            # cN48[p48] = (p%3)*N ; j48[p48] = p//3
            spi = pssm.tile([P, QCOLS], F32, tag="smallps")
            nc.tensor.matmul(spi[0:48, 0:1], wsel, cn3val, start=True, stop=True)
            cN48 = consts.tile([48, 1], F32)
            nc.scalar.copy(cN48, spi[0:48, 0:1])
            nc.tensor.matmul(spi[0:48, 8:9], wjsel, j16col, start=True, stop=True)
            j48 = consts.tile([48, 1], F32)
            nc.scalar.copy(j48, spi[0:48, 8:9])
            c48base = consts.tile([48, 1], F32)
            nc.vector.tensor_add(c48base, cN48, j48)

            # per-batch offset base constants
            jbN = []      # [16,1] : j + b*3N + N      (y plane)
            cn3b = []     # [3,1]  : c*N + b*3N        (centroid)
            c48b = []     # [48,1] : (p%3)*N + p//3 + b*3N  (dist rescue)
            for b in range(BPC):
                t1 = consts.tile([SEG, 1], F32, tag=f"jbN{b}")
                nc.vector.tensor_scalar_add(t1, j16col, float(b * 3 * N + N))
                jbN.append(t1)
                t2 = consts.tile([3, 1], F32, tag=f"cn3b{b}")
                nc.vector.tensor_scalar_add(t2, cn3val, float(b * 3 * N))
                cn3b.append(t2)
                t3 = consts.tile([48, 1], F32, tag=f"c48b{b}")
                nc.vector.tensor_scalar_add(t3, c48base, float(b * 3 * N))
                c48b.append(t3)

            negcrow_all = acc.tile([1, BPC, 3], F32)
            out_i = acc.tile([1, 2 * BPC], I32)

            # ---------------- DMA: interleaved y / xz per batch ----------------
            tys = []
            txzs = []
            for b in range(BPC):
                ty = ypool.tile([P, COLS], F32, tag="ty")
                tys.append(ty)
                nc.sync.dma_start(ty, xin[b, 1].rearrange("(p m) -> p m", p=P))
                txz = xzpool.tile([P, 2, COLS], F32, tag="txz")
                txzs.append(txz)
                nc.sync.dma_start(
                    txz, xin[b, 0::2].rearrange("c (p m) -> p c m", p=P)
                )

            # ---------------- per-batch phases ----------------
            negc128s = [None] * BPC

            def locate(tag, b, segmax, base_scalar_out, sp):
                """Exact two-level first-occurrence argmax over segmax [P, NSEG].
                Writes cell base (= p*2048 + s*16) into base_scalar_out [1,1]
                and returns (rows tile [2,P], M [1,1]) for reuse."""
                rowmax = finpool.tile([P, 1], F32, tag=f"rmax{tag}")
                nc.vector.tensor_reduce(rowmax, segmax, axis=AX, op=OP.max)
                candv = finpool.tile([P, NSEG], F32, tag=f"candv{tag}")
                nc.vector.scalar_tensor_tensor(
                    out=candv, in0=segmax, scalar=rowmax, in1=ws,
                    op0=OP.is_equal, op1=OP.mult,
                )
                scode = finpool.tile([P, 1], F32, tag=f"scode{tag}")
                nc.vector.tensor_reduce(scode, candv, axis=AX, op=OP.max)
                cbase = finpool.tile([P, 1], F32, tag=f"cb{tag}")
                nc.vector.tensor_add(cbase, scode, wpb)
                pstv = sp[0:1, 0:P]
                nc.tensor.transpose(pstv, rowmax, ident)
                pstc = sp[0:1, 192 : 192 + P]
                nc.tensor.transpose(pstc, cbase, ident)
                rowv = finpool.tile([1, P], F32, tag=f"rowv{tag}")
                nc.scalar.copy(rowv, pstv)
                rowc = finpool.tile([1, P], F32, tag=f"rowc{tag}")
                nc.scalar.copy(rowc, pstc)
                m = finpool.tile([1, 1], F32, tag=f"m{tag}")
                nc.vector.tensor_reduce(m, rowv, axis=AX, op=OP.max)
                candc = finpool.tile([1, P], F32, tag=f"candc{tag}")
                nc.vector.scalar_tensor_tensor(
                    out=candc, in0=rowv, scalar=m, in1=rowc,
                    op0=OP.is_equal, op1=OP.mult,
                )
                bcode = finpool.tile([1, 1], F32, tag=f"bcode{tag}")
                nc.vector.tensor_reduce(bcode, candc, axis=AX, op=OP.max)
                # base = N - bcode
                nc.scalar.activation(
                    base_scalar_out, bcode, IDENT, scale=-1.0, bias=cN_1
                )
                return m

            def scan16(tag, b, row16, base, out_col):
                """First-occurrence argmax over a [1,16] row (vs its own max);
                writes global index (base + j*) to out_i[0, out_col]."""
                rmax = finpool.tile([1, 1], F32, tag=f"rx{tag}")
                nc.vector.tensor_reduce(rmax, row16, axis=AX, op=OP.max)
                jcand = finpool.tile([1, SEG], F32, tag=f"jc{tag}")
                nc.vector.scalar_tensor_tensor(
                    out=jcand, in0=row16, scalar=rmax, in1=wj16,
                    op0=OP.is_equal, op1=OP.mult,
                )
                jcode = finpool.tile([1, 1], F32, tag=f"jq{tag}")
                nc.vector.tensor_reduce(jcode, jcand, axis=AX, op=OP.max)
                jstar = finpool.tile([1, 1], F32, tag=f"js{tag}")
                nc.scalar.activation(jstar, jcode, IDENT, scale=-1.0, bias=c16_1)
                gidx = finpool.tile([1, 1], F32, tag=f"gi{tag}")
                nc.vector.tensor_add(gidx, jstar, base)
                nc.scalar.copy(out_i[0:1, out_col : out_col + 1], gidx)
                return gidx

            def y_phase(b):
                ty = tys[b]
                sp = pssm.tile([P, QCOLS], F32, tag="smallps")
                yseg = segpool.tile([P, NSEG], F32, tag="yseg")
                nc.vector.tensor_reduce(
                    yseg, ty.rearrange("p (s j) -> p s j", j=SEG), axis=AX, op=OP.max
                )
                ybase = finpool.tile([1, 1], F32, tag="ybase")
                locate("y", b, yseg, ybase, sp)
                # gather the winning 16-run of the y plane
                psb16 = sp[0:SEG, 128:129]
                nc.tensor.matmul(psb16, ones16, ybase, start=True, stop=True)
                base16 = finpool.tile([SEG, 1], F32, tag="base16")
                nc.scalar.copy(base16, psb16)
                offs16 = finpool.tile([SEG, 1], U32, tag="offs16")
                nc.scalar.activation(offs16, jbN[b], IDENT, bias=base16)
                yg = finpool.tile([SEG, 1], F32, tag="yg")
                nc.gpsimd.indirect_dma_start(
                    out=yg, out_offset=None, in_=xflat,
                    in_offset=bass.IndirectOffsetOnAxis(ap=offs16, axis=0),
                )
                psr = sp[0:1, 136:152]
                nc.tensor.transpose(psr, yg, ident[0:SEG, 0:SEG])
                ysr = finpool.tile([1, SEG], F32, tag="ysr")
                nc.scalar.copy(ysr, psr)
                yidx = scan16("y", b, ysr, ybase, b)
                # centroid gather + negated broadcast bias
                psb3 = sp[0:3, 160:161]
                nc.tensor.matmul(psb3, ones3, yidx, start=True, stop=True)
                idx3 = finpool.tile([3, 1], F32, tag="idx3")
                nc.scalar.copy(idx3, psb3)
                coffs = finpool.tile([3, 1], U32, tag="coffs")
                nc.scalar.activation(coffs, cn3b[b], IDENT, bias=idx3)
                c3 = finpool.tile([3, 1], F32, tag="c3")
                nc.gpsimd.indirect_dma_start(
                    out=c3, out_offset=None, in_=xflat,
                    in_offset=bass.IndirectOffsetOnAxis(ap=coffs, axis=0),
                )
                negc3 = finpool.tile([3, 1], F32, tag="negc3")
                nc.scalar.activation(negc3, c3, IDENT, scale=-1.0, bias=zero3)
                psc = sp[0:1, 168:171]
                nc.tensor.transpose(psc, negc3, ident[0:3, 0:3])
                nc.scalar.copy(negcrow_all[0:1, b, :], psc)
                psnc = sp[:, 176:179]
                nc.tensor.matmul(
                    psnc, ones128, negcrow_all[0:1, b, :], start=True, stop=True
                )
                negc128 = finpool.tile([P, 3], F32, tag="negc128")
                nc.scalar.copy(negc128, psnc)
                negc128s[b] = negc128

            def dist_phase(b):
                ty, txz = tys[b], txzs[b]
                negc128 = negc128s[b]
                sqx = sqpool.tile([P, COLS], F32, tag="sqx")
                nc.scalar.activation(sqx, txz[:, 0], SQUARE, bias=negc128[:, 0:1])
                sqy = sqpool.tile([P, COLS], F32, tag="sqy")
                nc.scalar.activation(sqy, ty, SQUARE, bias=negc128[:, 1:2])
                sqz = sqpool.tile([P, COLS], F32, tag="sqz")
                nc.scalar.activation(sqz, txz[:, 1], SQUARE, bias=negc128[:, 2:3])
                psd = psd_pool.tile([P, COLS], F32, tag="psd")
                dseg = segpool.tile([P, NSEG], F32, tag="dseg")
                for q in range(NQ):
                    sl = slice(q * QCOLS, (q + 1) * QCOLS)
                    nc.tensor.matmul(psd[:, sl], ident, sqx[:, sl], start=True, stop=False)
                    nc.tensor.matmul(psd[:, sl], ident, sqy[:, sl], start=False, stop=False)
                    nc.tensor.matmul(psd[:, sl], ident, sqz[:, sl], start=False, stop=True)
                    nc.vector.tensor_reduce(
                        dseg[:, q * (QCOLS // SEG) : (q + 1) * (QCOLS // SEG)],
                        psd[:, sl].rearrange("p (s j) -> p s j", j=SEG),
                        axis=AX, op=OP.max,
                    )
                dbase = finpool.tile([1, 1], F32, tag="dbase")
                sp = pssm.tile([P, QCOLS], F32, tag="smallps")
                locate("d", b, dseg, dbase, sp)
                # gather 16 candidate points (x,y,z interleaved p=3j+c)
                psb48 = sp[0:48, 128:129]
                nc.tensor.matmul(psb48, ones48, dbase, start=True, stop=True)
                base48 = finpool.tile([48, 1], F32, tag="base48")
                nc.scalar.copy(base48, psb48)
                offs48 = finpool.tile([48, 1], U32, tag="offs48")
                nc.scalar.activation(offs48, c48b[b], IDENT, bias=base48)
                g48 = finpool.tile([48, 1], F32, tag="g48")
                nc.gpsimd.indirect_dma_start(
                    out=g48, out_offset=None, in_=xflat,
                    in_offset=bass.IndirectOffsetOnAxis(ap=offs48, axis=0),
                )
                psg = sp[0:1, 136:184]
                nc.tensor.transpose(psg, g48, ident[0:48, 0:48])
                g48r = finpool.tile([1, 48], F32, tag="g48r")
                nc.scalar.copy(g48r, psg)
                sub = finpool.tile([1, 16, 3], F32, tag="sub48")
                nc.vector.tensor_add(
                    sub,
                    g48r.rearrange("o (j c) -> o j c", c=3),
                    negcrow_all[0:1, b, :][:, None, :].to_broadcast([1, 16, 3]),
                )
                sq48 = finpool.tile([1, 16, 3], F32, tag="sq48")
                nc.vector.tensor_mul(sq48, sub, sub)
                d16 = finpool.tile([1, SEG], F32, tag="d16")
                nc.vector.tensor_reduce(d16, sq48, axis=AX, op=OP.add)
                scan16("d", b, d16, dbase, BPC + b)

            # software-pipelined emission: y0, y1, d0, y2, d1, ..., y7, d6, d7
            y_phase(0)
            for b in range(1, BPC):
                y_phase(b)
                dist_phase(b - 1)
            dist_phase(BPC - 1)

            nc.sync.dma_start(out[:, :], out_i[:, :])

    nc.compile()
    return nc


_NC_CACHE = None


def _get_nc():
    global _NC_CACHE
    if _NC_CACHE is None:
        _NC_CACHE = build_nc()
    return _NC_CACHE


def kernel(xyz: np.ndarray) -> np.ndarray:
    from concourse.bass_utils import run_bass_kernel_spmd

    assert xyz.shape == (1, B, 3, N), xyz.shape
    xyz = np.ascontiguousarray(xyz, dtype=np.float32)
    nc = _get_nc()
    in_maps = [
        {"xyz": np.ascontiguousarray(xyz[0, k * BPC : (k + 1) * BPC])}
        for k in range(N_CORES)
    ]
    res = run_bass_kernel_spmd(nc, in_maps, core_ids=list(range(N_CORES)))
    # out layout per core: [1, 16] = [idx0 x8 | idx1 x8]
    outs = [res.results[k]["idx"].reshape(2, BPC).T for k in range(N_CORES)]
    return np.concatenate(outs, axis=0).astype(np.int64)


# revision 30
# speedup vs baseline: 1.3855x; 1.3855x over previous
"""Farthest-point-sampling (npoint=2) Bass kernel for Trainium2 — v3.

Problem: xyz [1, 64, 3, 262144] fp32 -> indices [64, 2] (int64 on host).
Per batch b:
  idx0 = argmax_n y[n]            (y = coord plane 1)
  c    = (x,y,z)[idx0]
  idx1 = argmax_n ((x-cx)^2 + (y-cy)^2 + (z-cz)^2)
argmax = first occurrence on ties (jnp.argmax semantics).

Sharding: data-parallel over batch; 8 NeuronCores x 8 batches each.

v3 design (bit-exact vs fp32 reference end-to-end):
- Streaming: per-batch segmented VectorE max-reduce ([128,2048] ->
  [128,128] segment maxes of 16) for both the y plane and the distance.
- Distances: ScalarE Square(x + (-c)) (bit-exact), then the two adds are
  split by column range: TensorE identity-matmul accumulate for cols
  0:512 (PSUM, exact fp32) and GpSimd tensor_add for cols 512:2048.
- Argmax recovery: finales batched over groups of 4 batches — exact
  two-level first-occurrence locate (per-partition first-max-segment
  code, PE transpose, cross-partition base code), then one indirect-DMA
  gather of the winning 16-point runs and a single-partition scan.
- GpSimd uses only standard-library ops (tensor_tensor, iota) + DGE.
"""

import numpy as np

import concourse.bacc as bacc
import concourse.bass as bass
import concourse.mybir as mybir
from concourse.masks import make_identity
from concourse.tile import TileContext

B = 64  # full batch
N_CORES = 8
BPC = B // N_CORES  # batches per core
GRP = 4             # batches per finale group
NGRP = BPC // GRP
N = 262144
P = 128
COLS = N // P       # 2048
SEG = 16            # points per segment
NSEG = COLS // SEG  # 128 segments per partition
QCOLS = 512         # PE add range (cols 0:512); GpSimd takes 512:2048
GCOLS = COLS - QCOLS

F32 = mybir.dt.float32
U32 = mybir.dt.uint32
I32 = mybir.dt.int32
AX = mybir.AxisListType.X
OP = mybir.AluOpType
SQUARE = mybir.ActivationFunctionType.Square
IDENT = mybir.ActivationFunctionType.Identity


def build_nc():
    nc = bacc.Bacc()
    xin = nc.dram_tensor("xyz", [BPC, 3, N], F32, kind="ExternalInput")
    out = nc.dram_tensor("idx", [1, 2 * BPC], I32, kind="ExternalOutput")
    xflat = xin.rearrange("b c n -> (b c n)")[:, None]

    with TileContext(nc) as tc:
        with (
            tc.tile_pool(name="consts", bufs=1) as consts,
            tc.tile_pool(name="acc", bufs=1) as acc,
            tc.tile_pool(name="ypool", bufs=BPC) as ypool,
            tc.tile_pool(name="xzpool", bufs=3) as xzpool,
            tc.tile_pool(name="sqpool", bufs=2) as sqpool,
            tc.tile_pool(name="dpool", bufs=2) as dpool,
            tc.tile_pool(name="finpool", bufs=2) as finpool,
            tc.tile_pool(name="psd", bufs=2, space="PSUM") as psd_pool,
            tc.tile_pool(name="pssm", bufs=2, space="PSUM") as pssm,
        ):
            # ---------------- constants ----------------
            ident = consts.tile([P, P], F32)
            make_identity(nc, ident)

            def iota_f32(shape, tag, pattern, base, cm):
                ti = consts.tile(shape, I32, tag=tag + "_i")
                nc.gpsimd.iota(ti, pattern=pattern, base=base, channel_multiplier=cm)
                tf = consts.tile(shape, F32, tag=tag)
                nc.vector.tensor_copy(tf, ti)
                return tf

            # ws[p, s] = 2048 - 16*s ; wpb[p] = N - 2048 - 2048*p ; wj[j] = 16 - j
            ws = iota_f32([P, NSEG], "ws", [[-SEG, NSEG]], COLS, 0)
            wpb = iota_f32([P, 1], "wpb", [[0, 1]], N - COLS, -COLS)
            wj16 = iota_f32([1, SEG], "wj16", [[-1, SEG]], SEG, 0)
            cn3val = iota_f32([3, 1], "cn3val", [[0, 1]], 0, N)   # c*N
            bidx4 = iota_f32([GRP, 1], "bidx4", [[0, 1]], 0, 1)   # 0..3
            p64 = iota_f32([64, 1], "p64", [[0, 1]], 0, 1)        # 0..63
            pidx4 = iota_f32([GRP, 1], "pidx4", [[0, 1]], 0, 1)
            pidx3 = iota_f32([3, 1], "pidx3", [[0, 1]], 0, 1)
            # selector builders: f//16 on [4,64], f//3 and f%3 on [*,12]
            fdiv16 = iota_f32([GRP, GRP, SEG], "fdiv16", [[1, GRP], [0, SEG]], 0, 0)
            fdiv3_12 = iota_f32([GRP, GRP, 3], "fdiv3_12", [[1, GRP], [0, 3]], 0, 0)
            fm3_12 = iota_f32([3, GRP, 3], "fm3_12", [[0, GRP], [1, 3]], 0, 0)

            w16blk = consts.tile([GRP, 64], F32)   # W[bl, 16bl+j] = 1
            nc.vector.tensor_tensor(
                out=w16blk, in0=fdiv16.rearrange("p a b -> p (a b)"),
                in1=pidx4.to_broadcast([GRP, 64]), op=OP.is_equal,
            )
            w3blk = consts.tile([GRP, 12], F32)    # W[bl, 3bl+c] = 1
            nc.vector.tensor_tensor(
                out=w3blk, in0=fdiv3_12.rearrange("p a b -> p (a b)"),
                in1=pidx4.to_broadcast([GRP, 12]), op=OP.is_equal,
            )
            wc3 = consts.tile([3, 12], F32)        # W[c, 3bl+c] = 1
            nc.vector.tensor_tensor(
                out=wc3, in0=fm3_12.rearrange("p a b -> p (a b)"),
                in1=pidx3.to_broadcast([3, 12]), op=OP.is_equal,
            )

            ones16 = consts.tile([1, SEG], F32)
            nc.vector.memset(ones16, 1.0)
            ones128 = consts.tile([1, P], F32)
            nc.vector.memset(ones128, 1.0)
            cN_4 = consts.tile([GRP, 1], F32)
            nc.vector.memset(cN_4, float(N))
            c16_1 = consts.tile([1, 1], F32)
            nc.vector.memset(c16_1, float(SEG))
            c16_4 = consts.tile([GRP, 1], F32)
            nc.vector.memset(c16_4, float(SEG))
            zero12 = consts.tile([12, 1], F32)
            nc.vector.memset(zero12, 0.0)

            # --- init-time composed constants (via PE selector matmuls) ---
            spi = pssm.tile([P, QCOLS], F32, tag="smallps")
            # b64[p] = p//16 ; j64[p] = p%16
            nc.tensor.matmul(spi[0:64, 0:1], w16blk, bidx4, start=True, stop=True)
            b64 = consts.tile([64, 1], F32)
            nc.scalar.copy(b64, spi[0:64, 0:1])
            j64 = consts.tile([64, 1], F32)
            t64 = consts.tile([64, 1], F32)
            nc.vector.tensor_scalar_mul(t64, b64, float(SEG))
            nc.vector.tensor_sub(j64, p64, t64)
            # bl12[p] = p//3 ; cN12[p] = (p%3)*N
            nc.tensor.matmul(spi[0:12, 8:9], w3blk, bidx4, start=True, stop=True)
            bl12 = consts.tile([12, 1], F32)
            nc.scalar.copy(bl12, spi[0:12, 8:9])
            nc.tensor.matmul(spi[0:12, 16:17], wc3, cn3val, start=True, stop=True)
            cN12 = consts.tile([12, 1], F32)
            nc.scalar.copy(cN12, spi[0:12, 16:17])

            # consty64[g] = j + (4g+bl)*3N + N  (at p = 16*bl + j)
            # constd64[g][c] = consty64[g] + (c-1)*N
            # const12[g] = (p%3)*N + (4g+p//3)*3N
            y64c, d64c, c12c = [], [], []
            base_t = consts.tile([64, 1], F32, tag="base_t")
            nc.vector.tensor_scalar_mul(base_t, b64, float(3 * N))
            nc.vector.tensor_add(base_t, base_t, j64)
            c12_t = consts.tile([12, 1], F32, tag="c12_t")
            nc.vector.tensor_scalar_mul(c12_t, bl12, float(3 * N))
            nc.vector.tensor_add(c12_t, c12_t, cN12)
            for g in range(NGRP):
                ty64 = consts.tile([64, 1], F32, tag=f"y64c{g}")
                nc.vector.tensor_scalar_add(
                    ty64, base_t, float(4 * g * 3 * N + N)
                )
                y64c.append(ty64)
                row = []
                for c in range(3):
                    tdc = consts.tile([64, 1], F32, tag=f"d64c{g}_{c}")
                    nc.vector.tensor_scalar_add(tdc, ty64, float((c - 1) * N))
                    row.append(tdc)
                d64c.append(row)
                t12 = consts.tile([12, 1], F32, tag=f"c12c{g}")
                nc.vector.tensor_scalar_add(t12, c12_t, float(4 * g * 3 * N))
                c12c.append(t12)

            negcrow_all = acc.tile([1, BPC, 3], F32)
            negc128_all = acc.tile([P, BPC, 3], F32)
            ysegall = acc.tile([P, BPC, NSEG], F32)
            dsegall = acc.tile([P, BPC, NSEG], F32)
            out_i = acc.tile([1, 2 * BPC], I32)

            # ---------------- DMA ----------------
            # y planes for group 0 first, then interleave remaining.
            tys = [None] * BPC
            txzs = [None] * BPC

            def dma_y(b):
                ty = ypool.tile([P, COLS], F32, tag="ty")
                tys[b] = ty
                nc.sync.dma_start(ty, xin[b, 1].rearrange("(p m) -> p m", p=P))

            def dma_xz(b):
                txz = xzpool.tile([P, 2, COLS], F32, tag="txz")
                txzs[b] = txz
                nc.sync.dma_start(
                    txz, xin[b, 0::2].rearrange("c (p m) -> p c m", p=P)
                )

            # ---------------- building blocks ----------------
            def yseg_reduce(b):
                nc.vector.tensor_reduce(
                    ysegall[:, b, :],
                    tys[b].rearrange("p (s j) -> p s j", j=SEG),
                    axis=AX, op=OP.max,
                )

            def locate_group(tag, g, segall):
                """Exact first-occurrence argmax cell per batch in group g.
                Returns base4 [GRP,1] = p*2048 + s*16 per batch."""
                segs = segall[:, g * GRP : (g + 1) * GRP, :]  # [P, GRP, NSEG]
                rowmax = finpool.tile([P, GRP], F32, tag=f"rmax{tag}")
                nc.vector.tensor_reduce(rowmax, segs, axis=AX, op=OP.max)
                scode = finpool.tile([P, GRP], F32, tag=f"scode{tag}")
                for bl in range(GRP):
                    candv = finpool.tile([P, NSEG], F32, tag=f"candv{tag}")
                    nc.vector.scalar_tensor_tensor(
                        out=candv, in0=segs[:, bl, :],
                        scalar=rowmax[:, bl : bl + 1], in1=ws,
                        op0=OP.is_equal, op1=OP.mult,
                    )
                    nc.vector.tensor_reduce(
                        scode[:, bl : bl + 1], candv, axis=AX, op=OP.max
                    )
                cbase = finpool.tile([P, GRP], F32, tag=f"cb{tag}")
                nc.vector.tensor_add(
                    cbase, scode, wpb.to_broadcast([P, GRP])
                )
                sp = pssm.tile([P, QCOLS], F32, tag="smallps")
                nc.tensor.transpose(sp[0:GRP, 0:P], rowmax, ident)
                nc.tensor.transpose(sp[0:GRP, 128:256], cbase, ident)
                rowv = finpool.tile([GRP, P], F32, tag=f"rowv{tag}")
                nc.scalar.copy(rowv, sp[0:GRP, 0:P])
                rowc = finpool.tile([GRP, P], F32, tag=f"rowc{tag}")
                nc.scalar.copy(rowc, sp[0:GRP, 128:256])
                m4 = finpool.tile([GRP, 1], F32, tag=f"m{tag}")
                nc.vector.tensor_reduce(m4, rowv, axis=AX, op=OP.max)
                candc = finpool.tile([GRP, P], F32, tag=f"candc{tag}")
                nc.vector.scalar_tensor_tensor(
                    out=candc, in0=rowv, scalar=m4, in1=rowc,
                    op0=OP.is_equal, op1=OP.mult,
                )
                bcode = finpool.tile([GRP, 1], F32, tag=f"bcode{tag}")
                nc.vector.tensor_reduce(bcode, candc, axis=AX, op=OP.max)
                base4 = finpool.tile([GRP, 1], F32, tag=f"base{tag}")
                nc.scalar.activation(base4, bcode, IDENT, scale=-1.0, bias=cN_4)
                return base4, sp

            def gather_rows(tag, g, base4, const64, sp, k):
                """Broadcast base4 to 64 partitions, add const64 offsets,
                gather 64 values from HBM, transpose to a [1,64] row.
                k in 0..2 selects disjoint scratch regions in sp."""
                nc.tensor.matmul(
                    sp[0:64, 280 + k : 281 + k], w16blk, base4, start=True, stop=True
                )
                base64 = finpool.tile([64, 1], F32, tag=f"b64{tag}")
                nc.scalar.copy(base64, sp[0:64, 280 + k : 281 + k])
                offs = finpool.tile([64, 1], U32, tag=f"offs{tag}")
                nc.scalar.activation(offs, const64, IDENT, bias=base64)
                gv = finpool.tile([64, 1], F32, tag=f"gv{tag}")
                nc.gpsimd.indirect_dma_start(
                    out=gv, out_offset=None, in_=xflat,
                    in_offset=bass.IndirectOffsetOnAxis(ap=offs, axis=0),
                )
                nc.tensor.transpose(
                    sp[0:1, 288 + 64 * k : 352 + 64 * k], gv, ident[0:64, 0:64]
                )
                row = finpool.tile([1, 64], F32, tag=f"grow{tag}")
                nc.scalar.copy(row, sp[0:1, 288 + 64 * k : 352 + 64 * k])
                return row

            def scan_rows(tag, row64):
                """Per-batch first-occurrence argmax j* within each 16-run.
                row64: [1,64] viewed [1,GRP,SEG]. Returns jstar4 [1,GRP]."""
                v = row64.rearrange("o (b j) -> o b j", j=SEG)
                rmax = finpool.tile([1, GRP], F32, tag=f"srm{tag}")
                nc.vector.tensor_reduce(rmax, v, axis=AX, op=OP.max)
                jm = finpool.tile([1, GRP, SEG], F32, tag=f"sjm{tag}")
                nc.vector.tensor_tensor(
                    out=jm, in0=v,
                    in1=rmax[:, :, None].to_broadcast([1, GRP, SEG]),
                    op=OP.is_equal,
                )
                jc = finpool.tile([1, GRP, SEG], F32, tag=f"sjc{tag}")
                nc.vector.tensor_tensor(
                    out=jc, in0=jm,
                    in1=wj16[:, None, :].to_broadcast([1, GRP, SEG]),
                    op=OP.mult,
                )
                jcode = finpool.tile([1, GRP], F32, tag=f"sjq{tag}")
                nc.vector.tensor_reduce(jcode, jc, axis=AX, op=OP.max)
                jstar = finpool.tile([1, GRP], F32, tag=f"sjs{tag}")
                nc.scalar.activation(jstar, jcode, IDENT, scale=-1.0, bias=c16_1)
                return jstar

            def idx_col(tag, jstar4, base4, sp):
                """idx = base + jstar per batch, as a [GRP,1] column."""
                nc.tensor.transpose(sp[0:GRP, 484:485], jstar4, ident[0:1, 0:1])
                jcol = finpool.tile([GRP, 1], F32, tag=f"jcol{tag}")
                nc.scalar.copy(jcol, sp[0:GRP, 484:485])
                icol = finpool.tile([GRP, 1], F32, tag=f"icol{tag}")
                nc.vector.tensor_add(icol, jcol, base4)
                return icol

            def write_out(tag, icol, sp, out_off):
                nc.tensor.transpose(sp[0:1, 488:492], icol, ident[0:GRP, 0:GRP])
                irow = finpool.tile([1, GRP], F32, tag=f"irow{tag}")
                nc.scalar.copy(irow, sp[0:1, 488:492])
                nc.scalar.copy(out_i[0:1, out_off : out_off + GRP], irow)

            # ---------------- y finale (per group) ----------------
            def y_finale(g):
                base4, sp = locate_group("y", g, ysegall)
                yrow = gather_rows("y", g, base4, y64c[g], sp, 0)
                jstar4 = scan_rows("y", yrow)
                yidx = idx_col("y", jstar4, base4, sp)
                write_out("y", yidx, sp, g * GRP)
                # centroid gather: 12 offsets (bl,c) -> negc rows + 128-bcast
                nc.tensor.matmul(sp[0:12, 496:497], w3blk, yidx, start=True, stop=True)
                i12 = finpool.tile([12, 1], F32, tag="i12y")
                nc.scalar.copy(i12, sp[0:12, 496:497])
                coffs = finpool.tile([12, 1], U32, tag="co")
                nc.scalar.activation(coffs, c12c[g], IDENT, bias=i12)
                c12v = finpool.tile([12, 1], F32, tag="c12v")
                nc.gpsimd.indirect_dma_start(
                    out=c12v, out_offset=None, in_=xflat,
                    in_offset=bass.IndirectOffsetOnAxis(ap=coffs, axis=0),
                )
                negc12 = finpool.tile([12, 1], F32, tag="nc12")
                nc.scalar.activation(negc12, c12v, IDENT, scale=-1.0, bias=zero12)
                nc.tensor.transpose(sp[0:1, 498:510], negc12, ident[0:12, 0:12])
                nc.scalar.copy(
                    negcrow_all[0:1, g * GRP : (g + 1) * GRP, :], sp[0:1, 498:510]
                )
                nc.tensor.matmul(
                    sp[:, 268:280], ones128,
                    negcrow_all[0:1, g * GRP : (g + 1) * GRP, :].rearrange(
                        "o b c -> o (b c)"
                    ),
                    start=True, stop=True,
                )
                nc.scalar.copy(
                    negc128_all[:, g * GRP : (g + 1) * GRP, :].rearrange(
                        "p b c -> p (b c)"
                    ),
                    sp[:, 268:280],
                )

            # ---------------- distance per batch ----------------
            def dist_batch(b):
                ty, txz = tys[b], txzs[b]
                sqx = sqpool.tile([P, COLS], F32, tag="sqx")
                nc.scalar.activation(
                    sqx, txz[:, 0], SQUARE, bias=negc128_all[:, b, 0:1]
                )
                sqy = sqpool.tile([P, COLS], F32, tag="sqy")
                nc.scalar.activation(
                    sqy, ty, SQUARE, bias=negc128_all[:, b, 1:2]
                )
                sqz = sqpool.tile([P, COLS], F32, tag="sqz")
                nc.scalar.activation(
                    sqz, txz[:, 1], SQUARE, bias=negc128_all[:, b, 2:3]
                )
                # cols 0:512 on PE (exact fp32 accumulate in PSUM)
                psd = psd_pool.tile([P, QCOLS], F32, tag="psd")
                nc.tensor.matmul(psd, ident, sqx[:, 0:QCOLS], start=True, stop=False)
                nc.tensor.matmul(psd, ident, sqy[:, 0:QCOLS], start=False, stop=False)
                nc.tensor.matmul(psd, ident, sqz[:, 0:QCOLS], start=False, stop=True)
                nc.vector.tensor_reduce(
                    dsegall[:, b, 0 : QCOLS // SEG],
                    psd.rearrange("p (s j) -> p s j", j=SEG),
                    axis=AX, op=OP.max,
                )
                # cols 512:2048 on GpSimd (exact fp32 adds)
                dv = dpool.tile([P, GCOLS], F32, tag="dv")
                nc.gpsimd.tensor_add(dv, sqx[:, QCOLS:], sqy[:, QCOLS:])
                nc.gpsimd.tensor_add(dv, dv, sqz[:, QCOLS:])
                nc.vector.tensor_reduce(
                    dsegall[:, b, QCOLS // SEG : NSEG],
                    dv.rearrange("p (s j) -> p s j", j=SEG),
                    axis=AX, op=OP.max,
                )

            # ---------------- dist finale (per group) ----------------
            def d_finale(g):
                base4, sp = locate_group("d", g, dsegall)
                rows = []
                for c in range(3):
                    rows.append(
                        gather_rows(f"d{c}", g, base4, d64c[g][c], sp, c)
                    )
                d64 = None
                for c in range(3):
                    sub = finpool.tile([1, GRP, SEG], F32, tag="subc")
                    nc.vector.tensor_tensor(
                        out=sub, in0=rows[c].rearrange("o (b j) -> o b j", j=SEG),
                        in1=negcrow_all[0:1, g * GRP : (g + 1) * GRP, c][
                            :, :, None
                        ].to_broadcast([1, GRP, SEG]),
                        op=OP.add,
                    )
                    sq = finpool.tile([1, GRP, SEG], F32, tag="sqc")
                    nc.vector.tensor_mul(sq, sub, sub)
                    if d64 is None:
                        d64 = sq
                    else:
                        acc_t = finpool.tile([1, GRP, SEG], F32, tag="dacc")
                        nc.vector.tensor_add(acc_t, d64, sq)
                        d64 = acc_t
                jstar4 = scan_rows("d", d64.rearrange("o b j -> o (b j)"))
                didx = idx_col("d", jstar4, base4, sp)
                write_out("d", didx, sp, BPC + g * GRP)

            # ---------------- emission ----------------
            for b in range(GRP):
                dma_y(b)
            dma_xz(0)
            for b in range(GRP, BPC):
                dma_y(b)
                dma_xz(b - GRP + 1)
            for b in range(BPC - GRP + 1, BPC):
                dma_xz(b)

            for b in range(GRP):
                yseg_reduce(b)
            y_finale(0)
            for b in range(GRP, BPC):
                yseg_reduce(b)
            y_finale(1)
            for b in range(0, GRP):
                dist_batch(b)
            d_finale(0)
            for b in range(GRP, BPC):
                dist_batch(b)
            d_finale(1)

            nc.sync.dma_start(out[:, :], out_i[:, :])

    nc.compile()
    return nc


_NC_CACHE = None


def _get_nc():
    global _NC_CACHE
    if _NC_CACHE is None:
        _NC_CACHE = build_nc()
    return _NC_CACHE


def kernel(xyz: np.ndarray) -> np.ndarray:
    from concourse.bass_utils import run_bass_kernel_spmd

    assert xyz.shape == (1, B, 3, N), xyz.shape
    xyz = np.ascontiguousarray(xyz, dtype=np.float32)
    nc = _get_nc()
    in_maps = [
        {"xyz": np.ascontiguousarray(xyz[0, k * BPC : (k + 1) * BPC])}
        for k in range(N_CORES)
    ]
    res = run_bass_kernel_spmd(nc, in_maps, core_ids=list(range(N_CORES)))
    # out layout per core: [1, 16] = [idx0 x8 | idx1 x8]
    outs = [res.results[k]["idx"].reshape(2, BPC).T for k in range(N_CORES)]
    return np.concatenate(outs, axis=0).astype(np.int64)


# revision 31
# speedup vs baseline: 1.6221x; 1.1708x over previous
"""Farthest-point-sampling (npoint=2) Bass kernel for Trainium2 — v3.

Problem: xyz [1, 64, 3, 262144] fp32 -> indices [64, 2] (int64 on host).
Per batch b:
  idx0 = argmax_n y[n]            (y = coord plane 1)
  c    = (x,y,z)[idx0]
  idx1 = argmax_n ((x-cx)^2 + (y-cy)^2 + (z-cz)^2)
argmax = first occurrence on ties (jnp.argmax semantics).

Sharding: data-parallel over batch; 8 NeuronCores x 8 batches each.

v3 design (bit-exact vs fp32 reference end-to-end):
- Streaming: per-batch segmented VectorE max-reduce ([128,2048] ->
  [128,128] segment maxes of 16) for both the y plane and the distance.
- Distances: ScalarE Square(x + (-c)) (bit-exact), then the two adds are
  split by column range: TensorE identity-matmul accumulate for cols
  0:512 (PSUM, exact fp32) and GpSimd tensor_add for cols 512:2048.
- Argmax recovery: finales batched over groups of 4 batches — exact
  two-level first-occurrence locate (per-partition first-max-segment
  code, PE transpose, cross-partition base code), then one indirect-DMA
  gather of the winning 16-point runs and a single-partition scan.
- GpSimd uses only standard-library ops (tensor_tensor, iota) + DGE.
"""

import numpy as np

import concourse.bacc as bacc
import concourse.bass as bass
import concourse.mybir as mybir
from concourse.masks import make_identity
from concourse.tile import TileContext

B = 64  # full batch
N_CORES = 8
BPC = B // N_CORES  # batches per core
GRP = 4             # batches per finale group
NGRP = BPC // GRP
N = 262144
P = 128
COLS = N // P       # 2048
SEG = 16            # points per segment
NSEG = COLS // SEG  # 128 segments per partition
QCOLS = 512         # PE add range (cols 0:512); GpSimd takes 512:2048
GCOLS = COLS - QCOLS

F32 = mybir.dt.float32
U32 = mybir.dt.uint32
I32 = mybir.dt.int32
AX = mybir.AxisListType.X
OP = mybir.AluOpType
SQUARE = mybir.ActivationFunctionType.Square
IDENT = mybir.ActivationFunctionType.Identity


def build_nc():
    nc = bacc.Bacc()
    xin = nc.dram_tensor("xyz", [BPC, 3, N], F32, kind="ExternalInput")
    out = nc.dram_tensor("idx", [1, 2 * BPC], I32, kind="ExternalOutput")
    xflat = xin.rearrange("b c n -> (b c n)")[:, None]

    with TileContext(nc) as tc:
        with (
            tc.tile_pool(name="consts", bufs=1) as consts,
            tc.tile_pool(name="acc", bufs=1) as acc,
            tc.tile_pool(name="ypool", bufs=BPC) as ypool,
            tc.tile_pool(name="xzpool", bufs=3) as xzpool,
            tc.tile_pool(name="sqpool", bufs=2) as sqpool,
            tc.tile_pool(name="dpool", bufs=2) as dpool,
            tc.tile_pool(name="finpool", bufs=2) as finpool,
            tc.tile_pool(name="psd", bufs=2, space="PSUM") as psd_pool,
            tc.tile_pool(name="pssm", bufs=2, space="PSUM") as pssm,
        ):
            # ---------------- constants ----------------
            ident = consts.tile([P, P], F32)
            make_identity(nc, ident)

            def iota_f32(shape, tag, pattern, base, cm):
                ti = consts.tile(shape, I32, tag=tag + "_i")
                nc.gpsimd.iota(ti, pattern=pattern, base=base, channel_multiplier=cm)
                tf = consts.tile(shape, F32, tag=tag)
                nc.vector.tensor_copy(tf, ti)
                return tf

            # ws[p, s] = 2048 - 16*s ; wpb[p] = N - 2048 - 2048*p ; wj[j] = 16 - j
            ws = iota_f32([P, NSEG], "ws", [[-SEG, NSEG]], COLS, 0)
            wpb = iota_f32([P, 1], "wpb", [[0, 1]], N - COLS, -COLS)
            wpb2 = iota_f32([P, 1], "wpb2", [[0, 1]], N, -COLS)  # N - 2048p
            wj16 = iota_f32([1, SEG], "wj16", [[-1, SEG]], SEG, 0)
            cn3val = iota_f32([3, 1], "cn3val", [[0, 1]], 0, N)   # c*N
            bidx4 = iota_f32([GRP, 1], "bidx4", [[0, 1]], 0, 1)   # 0..3
            p64 = iota_f32([64, 1], "p64", [[0, 1]], 0, 1)        # 0..63
            pidx4 = iota_f32([GRP, 1], "pidx4", [[0, 1]], 0, 1)
            pidx3 = iota_f32([3, 1], "pidx3", [[0, 1]], 0, 1)
            # selector builders: f//16 on [4,64], f//3 and f%3 on [*,12]
            fdiv16 = iota_f32([GRP, GRP, SEG], "fdiv16", [[1, GRP], [0, SEG]], 0, 0)
            fdiv3_12 = iota_f32([GRP, GRP, 3], "fdiv3_12", [[1, GRP], [0, 3]], 0, 0)
            fm3_12 = iota_f32([3, GRP, 3], "fm3_12", [[0, GRP], [1, 3]], 0, 0)

            w16blk = consts.tile([GRP, 64], F32)   # W[bl, 16bl+j] = 1
            nc.vector.tensor_tensor(
                out=w16blk, in0=fdiv16.rearrange("p a b -> p (a b)"),
                in1=pidx4.to_broadcast([GRP, 64]), op=OP.is_equal,
            )
            w3blk = consts.tile([GRP, 12], F32)    # W[bl, 3bl+c] = 1
            nc.vector.tensor_tensor(
                out=w3blk, in0=fdiv3_12.rearrange("p a b -> p (a b)"),
                in1=pidx4.to_broadcast([GRP, 12]), op=OP.is_equal,
            )
            wc3 = consts.tile([3, 12], F32)        # W[c, 3bl+c] = 1
            nc.vector.tensor_tensor(
                out=wc3, in0=fm3_12.rearrange("p a b -> p (a b)"),
                in1=pidx3.to_broadcast([3, 12]), op=OP.is_equal,
            )

            ones16 = consts.tile([1, SEG], F32)
            nc.vector.memset(ones16, 1.0)
            ones128 = consts.tile([1, P], F32)
            nc.vector.memset(ones128, 1.0)
            cN_4 = consts.tile([GRP, 1], F32)
            nc.vector.memset(cN_4, float(N))
            c16_1 = consts.tile([1, 1], F32)
            nc.vector.memset(c16_1, float(SEG))
            c16_4 = consts.tile([GRP, 1], F32)
            nc.vector.memset(c16_4, float(SEG))
            zero12 = consts.tile([12, 1], F32)
            nc.vector.memset(zero12, 0.0)

            # --- init-time composed constants (via PE selector matmuls) ---
            spi = pssm.tile([P, QCOLS], F32, tag="smallps")
            # b64[p] = p//16 ; j64[p] = p%16
            nc.tensor.matmul(spi[0:64, 0:1], w16blk, bidx4, start=True, stop=True)
            b64 = consts.tile([64, 1], F32)
            nc.scalar.copy(b64, spi[0:64, 0:1])
            j64 = consts.tile([64, 1], F32)
            t64 = consts.tile([64, 1], F32)
            nc.vector.tensor_scalar_mul(t64, b64, float(SEG))
            nc.vector.tensor_sub(j64, p64, t64)
            # bl12[p] = p//3 ; cN12[p] = (p%3)*N
            nc.tensor.matmul(spi[0:12, 8:9], w3blk, bidx4, start=True, stop=True)
            bl12 = consts.tile([12, 1], F32)
            nc.scalar.copy(bl12, spi[0:12, 8:9])
            nc.tensor.matmul(spi[0:12, 16:17], wc3, cn3val, start=True, stop=True)
            cN12 = consts.tile([12, 1], F32)
            nc.scalar.copy(cN12, spi[0:12, 16:17])

            # consty64[g] = j + (4g+bl)*3N + N  (at p = 16*bl + j)
            # constd64[g][c] = consty64[g] + (c-1)*N
            # const12[g] = (p%3)*N + (4g+p//3)*3N
            y64c, d64c, c12c = [], [], []
            base_t = consts.tile([64, 1], F32, tag="base_t")
            nc.vector.tensor_scalar_mul(base_t, b64, float(3 * N))
            nc.vector.tensor_add(base_t, base_t, j64)
            c12_t = consts.tile([12, 1], F32, tag="c12_t")
            nc.vector.tensor_scalar_mul(c12_t, bl12, float(3 * N))
            nc.vector.tensor_add(c12_t, c12_t, cN12)
            for g in range(NGRP):
                ty64 = consts.tile([64, 1], F32, tag=f"y64c{g}")
                nc.vector.tensor_scalar_add(
                    ty64, base_t, float(4 * g * 3 * N + N)
                )
                y64c.append(ty64)
                row = []
                for c in range(3):
                    tdc = consts.tile([64, 1], F32, tag=f"d64c{g}_{c}")
                    nc.vector.tensor_scalar_add(tdc, ty64, float((c - 1) * N))
                    row.append(tdc)
                d64c.append(row)
                t12 = consts.tile([12, 1], F32, tag=f"c12c{g}")
                nc.vector.tensor_scalar_add(t12, c12_t, float(4 * g * 3 * N))
                c12c.append(t12)

            yvals = acc.tile([P, BPC], F32)
            ycb = acc.tile([P, BPC], F32)
            dvals = acc.tile([P, BPC], F32)
            dcb = acc.tile([P, BPC], F32)
            negcrow_all = acc.tile([1, BPC, 3], F32)
            negc128_all = acc.tile([P, BPC, 3], F32)
            ysegall = acc.tile([P, BPC, NSEG], F32)
            dsegall = acc.tile([P, BPC, NSEG], F32)
            out_i = acc.tile([1, 2 * BPC], I32)

            # ---------------- DMA ----------------
            # y planes for group 0 first, then interleave remaining.
            tys = [None] * BPC
            txzs = [None] * BPC

            def dma_y(b):
                ty = ypool.tile([P, COLS], F32, tag="ty")
                tys[b] = ty
                nc.sync.dma_start(ty, xin[b, 1].rearrange("(p m) -> p m", p=P))

            def dma_xz(b):
                txz = xzpool.tile([P, 2, COLS], F32, tag="txz")
                txzs[b] = txz
                nc.sync.dma_start(
                    txz, xin[b, 0::2].rearrange("c (p m) -> p c m", p=P)
                )

            # ---------------- building blocks ----------------
            def seg_argmax(b, segsrc, vals, cb):
                """Per-partition (max, first-seg) of segsrc [P,NSEG] ->
                vals[:, b] and cb[:, b] = N - 2048p - 16*s."""
                v8 = finpool.tile([P, 8], F32, tag="v8")
                nc.vector.max(out=v8, in_=segsrc)
                i8 = finpool.tile([P, 8], U32, tag="i8")
                nc.vector.max_index(i8, v8, segsrc)
                nc.vector.tensor_copy(vals[:, b : b + 1], v8[:, 0:1])
                i1f = finpool.tile([P, 1], F32, tag="i1f")
                nc.vector.tensor_copy(i1f, i8[:, 0:1])
                nc.vector.scalar_tensor_tensor(
                    out=cb[:, b : b + 1], in0=i1f, scalar=float(-SEG), in1=wpb2,
                    op0=OP.mult, op1=OP.add,
                )

            def yseg_reduce(b):
                nc.vector.tensor_reduce(
                    ysegall[:, b, :],
                    tys[b].rearrange("p (s j) -> p s j", j=SEG),
                    axis=AX, op=OP.max,
                )
                seg_argmax(b, ysegall[:, b, :], yvals, ycb)

            def locate_group(tag, g, vals, cb):
                """Exact first-occurrence argmax cell per batch in group g.
                Returns base4 [GRP,1] = p*2048 + s*16 per batch."""
                sp = pssm.tile([P, QCOLS], F32, tag="smallps")
                nc.tensor.transpose(
                    sp[0:GRP, 0:P], vals[:, g * GRP : (g + 1) * GRP], ident
                )
                nc.tensor.transpose(
                    sp[0:GRP, 128:256], cb[:, g * GRP : (g + 1) * GRP], ident
                )
                rowv = finpool.tile([GRP, P], F32, tag=f"rowv{tag}")
                nc.vector.tensor_copy(rowv, sp[0:GRP, 0:P])
                rowc = finpool.tile([GRP, P], F32, tag=f"rowc{tag}")
                nc.vector.tensor_copy(rowc, sp[0:GRP, 128:256])
                m4 = finpool.tile([GRP, 1], F32, tag=f"m{tag}")
                nc.vector.tensor_reduce(m4, rowv, axis=AX, op=OP.max)
                candc = finpool.tile([GRP, P], F32, tag=f"candc{tag}")
                nc.vector.scalar_tensor_tensor(
                    out=candc, in0=rowv, scalar=m4, in1=rowc,
                    op0=OP.is_equal, op1=OP.mult,
                )
                bcode = finpool.tile([GRP, 1], F32, tag=f"bcode{tag}")
                nc.vector.tensor_reduce(bcode, candc, axis=AX, op=OP.max)
                base4 = finpool.tile([GRP, 1], F32, tag=f"base{tag}")
                nc.vector.tensor_scalar(
                    out=base4, in0=bcode, scalar1=-1.0, scalar2=float(N),
                    op0=OP.mult, op1=OP.add,
                )
                return base4, sp

            def gather_rows(tag, g, base4, const64, sp, k):
                """Broadcast base4 to 64 partitions, add const64 offsets,
                gather 64 values from HBM, transpose to a [1,64] row.
                k in 0..2 selects disjoint scratch regions in sp."""
                nc.tensor.matmul(
                    sp[0:64, 280 + k : 281 + k], w16blk, base4, start=True, stop=True
                )
                base64 = finpool.tile([64, 1], F32, tag=f"b64{tag}")
                nc.vector.tensor_copy(base64, sp[0:64, 280 + k : 281 + k])
                offs = finpool.tile([64, 1], U32, tag=f"offs{tag}")
                nc.vector.tensor_add(offs, const64, base64)
                gv = finpool.tile([64, 1], F32, tag=f"gv{tag}")
                nc.gpsimd.indirect_dma_start(
                    out=gv, out_offset=None, in_=xflat,
                    in_offset=bass.IndirectOffsetOnAxis(ap=offs, axis=0),
                )
                nc.tensor.transpose(
                    sp[0:1, 288 + 64 * k : 352 + 64 * k], gv, ident[0:64, 0:64]
                )
                row = finpool.tile([1, 64], F32, tag=f"grow{tag}")
                nc.vector.tensor_copy(row, sp[0:1, 288 + 64 * k : 352 + 64 * k])
                return row

            def scan_rows(tag, row64):
                """Per-batch first-occurrence argmax j* within each 16-run.
                row64: [1,64] viewed [1,GRP,SEG]. Returns jstar4 [1,GRP]."""
                v = row64.rearrange("o (b j) -> o b j", j=SEG)
                rmax = finpool.tile([1, GRP], F32, tag=f"srm{tag}")
                nc.vector.tensor_reduce(rmax, v, axis=AX, op=OP.max)
                jm = finpool.tile([1, GRP, SEG], F32, tag=f"sjm{tag}")
                nc.vector.tensor_tensor(
                    out=jm, in0=v,
                    in1=rmax[:, :, None].to_broadcast([1, GRP, SEG]),
                    op=OP.is_equal,
                )
                jc = finpool.tile([1, GRP, SEG], F32, tag=f"sjc{tag}")
                nc.vector.tensor_tensor(
                    out=jc, in0=jm,
                    in1=wj16[:, None, :].to_broadcast([1, GRP, SEG]),
                    op=OP.mult,
                )
                jcode = finpool.tile([1, GRP], F32, tag=f"sjq{tag}")
                nc.vector.tensor_reduce(jcode, jc, axis=AX, op=OP.max)
                jstar = finpool.tile([1, GRP], F32, tag=f"sjs{tag}")
                nc.vector.tensor_scalar(
                    out=jstar, in0=jcode, scalar1=-1.0, scalar2=float(SEG),
                    op0=OP.mult, op1=OP.add,
                )
                return jstar

            def idx_col(tag, jstar4, base4, sp):
                """idx = base + jstar per batch, as a [GRP,1] column."""
                nc.tensor.transpose(sp[0:GRP, 484:485], jstar4, ident[0:1, 0:1])
                jcol = finpool.tile([GRP, 1], F32, tag=f"jcol{tag}")
                nc.vector.tensor_copy(jcol, sp[0:GRP, 484:485])
                icol = finpool.tile([GRP, 1], F32, tag=f"icol{tag}")
                nc.vector.tensor_add(icol, jcol, base4)
                return icol

            def write_out(tag, icol, sp, out_off):
                nc.tensor.transpose(sp[0:1, 488:492], icol, ident[0:GRP, 0:GRP])
                irow = finpool.tile([1, GRP], F32, tag=f"irow{tag}")
                nc.vector.tensor_copy(irow, sp[0:1, 488:492])
                nc.scalar.copy(out_i[0:1, out_off : out_off + GRP], irow)

            # ---------------- y finale (per group) ----------------
            def y_finale(g):
                base4, sp = locate_group("y", g, yvals, ycb)
                yrow = gather_rows("y", g, base4, y64c[g], sp, 0)
                jstar4 = scan_rows("y", yrow)
                yidx = idx_col("y", jstar4, base4, sp)
                write_out("y", yidx, sp, g * GRP)
                # centroid gather: 12 offsets (bl,c) -> negc rows + 128-bcast
                nc.tensor.matmul(sp[0:12, 496:497], w3blk, yidx, start=True, stop=True)
                i12 = finpool.tile([12, 1], F32, tag="i12y")
                nc.vector.tensor_copy(i12, sp[0:12, 496:497])
                coffs = finpool.tile([12, 1], U32, tag="co")
                nc.vector.tensor_add(coffs, c12c[g], i12)
                c12v = finpool.tile([12, 1], F32, tag="c12v")
                nc.gpsimd.indirect_dma_start(
                    out=c12v, out_offset=None, in_=xflat,
                    in_offset=bass.IndirectOffsetOnAxis(ap=coffs, axis=0),
                )
                negc12 = finpool.tile([12, 1], F32, tag="nc12")
                nc.vector.tensor_scalar_mul(negc12, c12v, -1.0)
                nc.tensor.transpose(sp[0:1, 498:510], negc12, ident[0:12, 0:12])
                nc.vector.tensor_copy(
                    negcrow_all[0:1, g * GRP : (g + 1) * GRP, :], sp[0:1, 498:510]
                )
                nc.tensor.matmul(
                    sp[:, 268:280], ones128,
                    negcrow_all[0:1, g * GRP : (g + 1) * GRP, :].rearrange(
                        "o b c -> o (b c)"
                    ),
                    start=True, stop=True,
                )
                nc.scalar.copy(
                    negc128_all[:, g * GRP : (g + 1) * GRP, :].rearrange(
                        "p b c -> p (b c)"
                    ),
                    sp[:, 268:280],
                )

            # ---------------- distance per batch ----------------
            def dist_batch(b):
                ty, txz = tys[b], txzs[b]
                sqx = sqpool.tile([P, COLS], F32, tag="sqx")
                nc.scalar.activation(
                    sqx, txz[:, 0], SQUARE, bias=negc128_all[:, b, 0:1]
                )
                sqy = sqpool.tile([P, COLS], F32, tag="sqy")
                nc.scalar.activation(
                    sqy, ty, SQUARE, bias=negc128_all[:, b, 1:2]
                )
                sqz = sqpool.tile([P, COLS], F32, tag="sqz")
                nc.scalar.activation(
                    sqz, txz[:, 1], SQUARE, bias=negc128_all[:, b, 2:3]
                )
                # cols 0:512 on PE (exact fp32 accumulate in PSUM)
                psd = psd_pool.tile([P, QCOLS], F32, tag="psd")
                nc.tensor.matmul(psd, ident, sqx[:, 0:QCOLS], start=True, stop=False)
                nc.tensor.matmul(psd, ident, sqy[:, 0:QCOLS], start=False, stop=False)
                nc.tensor.matmul(psd, ident, sqz[:, 0:QCOLS], start=False, stop=True)
                nc.vector.tensor_reduce(
                    dsegall[:, b, 0 : QCOLS // SEG],
                    psd.rearrange("p (s j) -> p s j", j=SEG),
                    axis=AX, op=OP.max,
                )
                # cols 512:2048 on GpSimd (exact fp32 adds)
                dv = dpool.tile([P, GCOLS], F32, tag="dv")
                nc.gpsimd.tensor_add(dv, sqx[:, QCOLS:], sqy[:, QCOLS:])
                nc.gpsimd.tensor_add(dv, dv, sqz[:, QCOLS:])
                nc.vector.tensor_reduce(
                    dsegall[:, b, QCOLS // SEG : NSEG],
                    dv.rearrange("p (s j) -> p s j", j=SEG),
                    axis=AX, op=OP.max,
                )
                seg_argmax(b, dsegall[:, b, :], dvals, dcb)

            # ---------------- dist finale (per group) ----------------
            def d_finale(g):
                base4, sp = locate_group("d", g, dvals, dcb)
                rows = []
                for c in range(3):
                    rows.append(
                        gather_rows(f"d{c}", g, base4, d64c[g][c], sp, c)
                    )
                d64 = None
                for c in range(3):
                    sub = finpool.tile([1, GRP, SEG], F32, tag="subc")
                    nc.vector.tensor_tensor(
                        out=sub, in0=rows[c].rearrange("o (b j) -> o b j", j=SEG),
                        in1=negcrow_all[0:1, g * GRP : (g + 1) * GRP, c][
                            :, :, None
                        ].to_broadcast([1, GRP, SEG]),
                        op=OP.add,
                    )
                    sq = finpool.tile([1, GRP, SEG], F32, tag="sqc")
                    nc.vector.tensor_mul(sq, sub, sub)
                    if d64 is None:
                        d64 = sq
                    else:
                        acc_t = finpool.tile([1, GRP, SEG], F32, tag="dacc")
                        nc.vector.tensor_add(acc_t, d64, sq)
                        d64 = acc_t
                jstar4 = scan_rows("d", d64.rearrange("o b j -> o (b j)"))
                didx = idx_col("d", jstar4, base4, sp)
                write_out("d", didx, sp, BPC + g * GRP)

            # ---------------- emission ----------------
            for b in range(BPC):
                dma_y(b)
            for b in range(BPC):
                dma_xz(b)

            for b in range(GRP):
                yseg_reduce(b)
            y_finale(0)
            for b in range(GRP, BPC):
                yseg_reduce(b)
            y_finale(1)
            for b in range(0, GRP):
                dist_batch(b)
            d_finale(0)
            for b in range(GRP, BPC):
                dist_batch(b)
            d_finale(1)

            nc.sync.dma_start(out[:, :], out_i[:, :])

    nc.compile()
    return nc


_NC_CACHE = None


def _get_nc():
    global _NC_CACHE
    if _NC_CACHE is None:
        _NC_CACHE = build_nc()
    return _NC_CACHE


def kernel(xyz: np.ndarray) -> np.ndarray:
    from concourse.bass_utils import run_bass_kernel_spmd

    assert xyz.shape == (1, B, 3, N), xyz.shape
    xyz = np.ascontiguousarray(xyz, dtype=np.float32)
    nc = _get_nc()
    in_maps = [
        {"xyz": np.ascontiguousarray(xyz[0, k * BPC : (k + 1) * BPC])}
        for k in range(N_CORES)
    ]
    res = run_bass_kernel_spmd(nc, in_maps, core_ids=list(range(N_CORES)))
    # out layout per core: [1, 16] = [idx0 x8 | idx1 x8]
    outs = [res.results[k]["idx"].reshape(2, BPC).T for k in range(N_CORES)]
    return np.concatenate(outs, axis=0).astype(np.int64)
